# revision 11
# baseline (speedup 1.0000x reference)
"""Distributed causal multi-head attention block (GPT-2 style) for 8 TRN2 NeuronCores.

Sharding: data-parallel over batch (4 groups of 2 cores) x tensor-parallel over
heads (2 groups of 8 heads). Core c handles batch c//2, head-group c%2.

v3: host pre-transposes/casts inputs to bf16 (xT, wqkv, wproj) and pre-shapes
the biases; the 16 input-chunk DMAs are first on the sync queue so the first
qkv wave (8 PSUM banks, emitted kc-major) tracks DMA arrival from ~2us and
warms the HAM clock gate early. Softmax normalization broadcasts the
reciprocal on GpSimd (partition_broadcast) instead of a PE rank-1 matmul,
removing a PE head-of-line stall per head. DMA traffic is split: input
chunks + gathered-aT reloads on the sync queue, biases + collective staging
on the Pool queue, so late-head staging never blocks early reloads.

Per-core pipeline (all matmuls bf16 with f32 PSUM accumulation):
  1. qT,kT = W^T chunks @ xT (feat-major), v = xT^T-chunks @ Wv (S-major)
  2. per head: scores^T tiles = kT_h^T-slices @ qT_h (causally skipped),
     P^T = exp(scores/8) (+ triangular mask on diagonal blocks),
     a[q,65] = P^T-blocks^T @ [v_h | ones]  -> denominator in col 64,
     normalize rows by 1/denom -> aT_loc bf16 [FEAT, S]
  3. pair AllGather of aT_loc chunks -> full a for the batch
  4. c_proj half-columns: out[q,512] = aT-chunks^T @ Wproj_half + bias
Host assembles out[b, :, hg*512:(hg+1)*512] from each core.
"""

import numpy as np
import ml_dtypes

import concourse.bass as bass
import concourse.mybir as mybir
import concourse.tile as tile
from concourse import bacc
from concourse.bass_utils import run_bass_kernel_spmd
from concourse.masks import make_upper_triangular

F32 = mybir.dt.float32
BF16 = mybir.dt.bfloat16
AF = mybir.ActivationFunctionType
ALU = mybir.AluOpType

P = 128
S = 1024          # sequence length
NX = 1024         # model width
D = 64            # head dim
H_LOC = 8         # heads per core
FEAT = H_LOC * D  # 512 local attention features
NKC = NX // P     # 8 contraction chunks
NST = S // P      # 8 sequence tiles
VW = D + 1        # v block width incl. ones column (65)


def build():
    nc = bacc.Bacc(num_devices=8)
    xT = nc.dram_tensor("xT", [NX, S], BF16, kind="ExternalInput")
    wqkv = nc.dram_tensor("wqkv", [NX, 3 * FEAT], BF16, kind="ExternalInput")
    bqk_t = nc.dram_tensor("bqk_t", [P, 8], F32, kind="ExternalInput")
    bv_r = nc.dram_tensor("bv_r", [1, FEAT], BF16, kind="ExternalInput")
    wproj = nc.dram_tensor("wproj", [NX, FEAT], BF16, kind="ExternalInput")
    bp_r = nc.dram_tensor("bp_r", [1, FEAT], BF16, kind="ExternalInput")
    out = nc.dram_tensor("out", [S, FEAT], F32, kind="ExternalOutput")

    with tile.TileContext(nc) as tc:
        with (
            tc.tile_pool(name="pt", bufs=16) as ptp,           # P^T blocks
            tc.tile_pool(name="small", bufs=4) as small,       # recip vectors
            tc.tile_pool(name="outp", bufs=3) as outp,         # out f32 tiles
            tc.tile_pool(name="dram", bufs=1, space="DRAM") as dram,
            tc.tile_pool(name="resident", bufs=1) as res,
        ):
            # ---- resident SBUF tensors (distinct tags -> distinct slots) ----
            xT_all = res.tile([P, NKC * S], BF16, tag="xT_all")          # [NX, S] chunked
            wqkv_sb = res.tile([P, NKC * 3 * FEAT], BF16, tag="wqkv_sb")
            qkT_all = res.tile([P, 8 * S], BF16, tag="qkT_all")          # qT(0..3)|kT(4..7)
            v_sb = res.tile([P, NST * H_LOC * VW], BF16, tag="v_sb")
            aT_loc = res.tile([P, 4 * S], BF16, tag="aT_loc")            # [FEAT, S] chunked
            wp_sb = res.tile([P, NKC * FEAT], BF16, tag="wp_sb")
            aT_all = res.tile([P, 16 * FEAT], BF16, tag="aT_all")        # (qh,fc) stage-3 lhsT
            bias_sb = res.tile([P, 8], F32, tag="bias_sb")
            bv_row = res.tile([1, FEAT], BF16, tag="bv_row")
            bp_row = res.tile([1, FEAT], BF16, tag="bp_row")
            ones_row = res.tile([1, P], BF16, tag="ones_row")
            utri = res.tile([P, P], BF16, tag="utri")

            nc.vector.memset(ones_row[:], 1.0)
            make_upper_triangular(nc, utri[:], val=1.0, diag=True)
            nc.vector.memset(v_sb[:], 1.0)

            # ---- input stream. sync queue: the 16 x/w chunks first (the
            # critical path), then wproj, then gathered-aT reloads, then the
            # qh=1 output tiles. Pool queue: tiny biases first, then per-head
            # broadcasts / collective staging in head order. ----
            for kc in range(NKC):
                xs = slice(kc * P, (kc + 1) * P)
                if kc < 2:
                    # split the first chunks into need-aligned pieces so the
                    # first qkv matmuls start as soon as ~200 KB has landed
                    nc.sync.dma_start(
                        wqkv_sb[:, kc * 3 * FEAT : kc * 3 * FEAT + 640],
                        wqkv[xs, 0:640],
                    )
                    nc.sync.dma_start(
                        xT_all[:, kc * S : kc * S + 512], xT[xs, 0:512]
                    )
                    nc.sync.dma_start(
                        xT_all[:, kc * S + 512 : (kc + 1) * S], xT[xs, 512:1024]
                    )
                    nc.sync.dma_start(
                        wqkv_sb[:, kc * 3 * FEAT + 640 : (kc + 1) * 3 * FEAT],
                        wqkv[xs, 640:1536],
                    )
                else:
                    nc.sync.dma_start(xT_all[:, kc * S : (kc + 1) * S], xT[xs, :])
                    nc.sync.dma_start(
                        wqkv_sb[:, kc * 3 * FEAT : (kc + 1) * 3 * FEAT],
                        wqkv[xs, :],
                    )
            nc.gpsimd.dma_start(bias_sb[:], bqk_t[:, :])
            nc.gpsimd.dma_start(bv_row[:], bv_r[:, :])
            nc.gpsimd.dma_start(bp_row[:], bp_r[:, :])
            for fc in range(NKC):
                nc.sync.dma_start(
                    wp_sb[:, fc * FEAT : (fc + 1) * FEAT],
                    wproj[fc * P : (fc + 1) * P, :],
                )

            # ---- qkv group helpers ----
            # qk group (ft, half): psum [128f, 512s]; ft 0..3 = q, 4..7 = k
            def qk_mm(ps, ft, half, kc):
                nc.tensor.matmul(
                    ps[:],
                    wqkv_sb[:, kc * 3 * FEAT + ft * P : kc * 3 * FEAT + (ft + 1) * P],
                    xT_all[:, kc * S + half * 512 : kc * S + (half + 1) * 512],
                    start=(kc == 0),
                    stop=(kc == NKC - 1),
                )

            def qk_consume(ps, ft, half):
                # bias-add + bf16 cast on DVE
                nc.vector.tensor_scalar_add(
                    out=qkT_all[:, ft * S + half * 512 : ft * S + (half + 1) * 512],
                    in0=ps[:],
                    scalar1=bias_sb[:, ft : ft + 1],
                )

            # v group (st): psum [128s, 512d]
            def v_bias(ps):
                nc.tensor.matmul(ps[:], ones_row[:, 0:P], bv_row[:], start=True, stop=False)

            def v_mm(ps, st, kc):
                nc.tensor.matmul(
                    ps[:],
                    xT_all[:, kc * S + st * P : kc * S + (st + 1) * P],
                    wqkv_sb[:, kc * 3 * FEAT + 1024 : kc * 3 * FEAT + 1536],
                    start=False,
                    stop=(kc == NKC - 1),
                )

            def v_consume(ps, st):
                # strided copy: 8 head blocks of 64 cols into the 65-wide
                # (v | ones) layout in one DVE instruction
                base = st * H_LOC * VW
                dst = v_sb[:, base : base + H_LOC * VW].rearrange(
                    "p (h w) -> p h w", h=H_LOC
                )[:, :, 0:D]
                src = ps[:].rearrange("p (h d) -> p h d", h=H_LOC)
                nc.vector.tensor_copy(out=dst, in_=src)

            # ---- gather plumbing (pair AllGather of aT_loc) ----
            PAIRS = [[0, 1], [2, 3], [4, 5], [6, 7]]
            cc_in0 = dram.tile([FEAT, 512], BF16, name="cc_in0")
            cc_out0 = dram.tile([2 * FEAT, 512], BF16, name="cc_out0")
            FT_PARTS = [[0, 1], [2], [3]]
            cc_in1 = [
                dram.tile([len(fts) * P, 512], BF16, name=f"cc_in1{i}")
                for i, fts in enumerate(FT_PARTS)
            ]
            cc_out1 = [
                dram.tile([2 * len(fts) * P, 512], BF16, name=f"cc_out1{i}")
                for i, fts in enumerate(FT_PARTS)
            ]

            def gather_half0():
                for ft in range(4):
                    nc.gpsimd.dma_start(
                        cc_in0[ft * P : (ft + 1) * P, :],
                        aT_loc[:, ft * S : ft * S + 512],
                    )
                nc.gpsimd.collective_compute(
                    "AllGather", ALU.bypass, replica_groups=PAIRS,
                    ins=[cc_in0[:].opt()], outs=[cc_out0[:].opt()],
                )

            def gather_half1(part):
                for i, ft in enumerate(FT_PARTS[part]):
                    nc.gpsimd.dma_start(
                        cc_in1[part][i * P : (i + 1) * P, :],
                        aT_loc[:, ft * S + 512 : (ft + 1) * S],
                    )
                nc.gpsimd.collective_compute(
                    "AllGather", ALU.bypass, replica_groups=PAIRS,
                    ins=[cc_in1[part][:].opt()], outs=[cc_out1[part][:].opt()],
                )

            def _gathered_src(qh2, fc):
                # global feature chunk fc: rank block fc//4, local ft fc%4
                blk, lft = fc // 4, fc % 4
                if qh2 == 0:
                    return cc_out0[(blk * 4 + lft) * P : (blk * 4 + lft + 1) * P, :]
                part = 0 if lft < 2 else lft - 1
                i = lft if lft < 2 else 0
                n = len(FT_PARTS[part])
                return cc_out1[part][(blk * n + i) * P : (blk * n + i + 1) * P, :]

            def proj_load(qh2, fcs=None):
                # reload gathered aT on the sync queue (nothing early sits
                # behind these; staging DMAs live on the Pool queue)
                for fc in fcs or range(NKC):
                    nc.sync.dma_start(
                        aT_all[:, (qh2 * NKC + fc) * FEAT : (qh2 * NKC + fc + 1) * FEAT],
                        _gathered_src(qh2, fc),
                    )

            # ---- wave 1: 8 groups fed in DMA-arrival order (kc-major) ----
            # groups: qk(0,0), qk(0,1), qk(4,0), qk(4,1), v(0..3) — exactly the
            # inputs heads 0/1 of q-half 0 need first
            W1_QK = [(0, 0), (0, 1), (4, 0), (4, 1)]
            W1_V = [0, 1, 2, 3]
            with tc.tile_pool(name="ps_w1", bufs=8, space="PSUM") as psw:
                w1ps = {}
                for ft, half in W1_QK:
                    w1ps[("qk", ft, half)] = psw.tile(
                        [P, 512], F32, name=f"w1qk{ft}{half}", tag="w1"
                    )
                for st in W1_V:
                    ps = psw.tile([P, 512], F32, name=f"w1v{st}", tag="w1")
                    w1ps[("v", st)] = ps
                    v_bias(ps)
                for kc in range(NKC):
                    # halves adjacent: q(0) then k(4) match first-chunk pieces
                    for ft, half in [(0, 0), (4, 0), (0, 1), (4, 1)]:
                        qk_mm(w1ps[("qk", ft, half)], ft, half, kc)
                    for st in W1_V:
                        v_mm(w1ps[("v", st)], st, kc)
                for ft, half in W1_QK:
                    qk_consume(w1ps[("qk", ft, half)], ft, half)
                for st in W1_V:
                    v_consume(w1ps[("v", st)], st)

            # ---- attention phase (with remaining qkv tiles interleaved) ----
            with (
                tc.tile_pool(name="ps_big", bufs=3, space="PSUM") as ps_big,
                tc.tile_pool(name="ps_sc", bufs=3, space="PSUM") as ps_sc,
            ):
                # later qkv groups: SBUF-fed, rotate through ps_big
                def qkT_tile(ft):
                    for half in range(2):
                        ps = ps_big.tile([P, 512], F32, name="ps_qk", tag="big")
                        for kc in range(NKC):
                            qk_mm(ps, ft, half, kc)
                        qk_consume(ps, ft, half)

                def v_tile(st):
                    ps = ps_big.tile([P, 512], F32, name="ps_v", tag="big")
                    v_bias(ps)
                    for kc in range(NKC):
                        v_mm(ps, st, kc)
                    v_consume(ps, st)

                def attention_head(qh, h):
                    nj = 4 * qh + 4                    # causal k-tiles for this half
                    prow = (h % 2) * D
                    qcol = (h // 2) * S            # qT feature-tile col base
                    kcol = (4 + h // 2) * S        # kT feature-tile col base
                    pt_blocks = []
                    for j in range(nj):
                        dloc = j - 4 * qh          # diagonal block index in this half
                        coff = max(dloc, 0) * P    # first allowed local q col
                        ps = ps_sc.tile([P, 512], F32)
                        ptb = ptp.tile([P, 512], BF16, tag="pt")
                        nc.tensor.matmul(
                            ps[:, coff:512],
                            qkT_all[prow : prow + D, kcol + j * P : kcol + (j + 1) * P],
                            qkT_all[
                                prow : prow + D,
                                qcol + qh * 512 + coff : qcol + (qh + 1) * 512,
                            ],
                            start=True,
                            stop=True,
                        )
                        nc.scalar.activation(
                            out=ptb[:, coff:512],
                            in_=ps[:, coff:512],
                            func=AF.Exp,
                            scale=0.125,
                        )
                        if dloc >= 0:
                            nc.vector.tensor_tensor(
                                out=ptb[:, coff : coff + P],
                                in0=ptb[:, coff : coff + P],
                                in1=utri[:],
                                op=ALU.mult,
                            )
                        pt_blocks.append((ptb, coff))
                    # aT[d, q] for this (head, half) + denominator row via ones
                    # col; each k-block only contributes to its causal q cols
                    psa = ps_sc.tile([VW, 512], F32, tag="psaT", bufs=2)
                    for j in range(nj):
                        ptb, coff = pt_blocks[j]
                        nc.tensor.matmul(
                            psa[:, coff:512],
                            v_sb[:, j * H_LOC * VW + h * VW : j * H_LOC * VW + (h + 1) * VW],
                            ptb[:, coff:512],
                            start=(j == 0),
                            stop=(j == nj - 1),
                        )
                    # fast-recip the denominator row, broadcast it down 64
                    # partitions on GpSimd, normalize straight out of PSUM
                    acols = slice((h // 2) * S + qh * 512, (h // 2) * S + (qh + 1) * 512)
                    db = small.tile([1, 512], F32, tag="db")
                    nc.vector.tensor_copy(out=db[:], in_=psa[D : D + 1, :])
                    rc = small.tile([1, 512], F32, tag="rc")
                    nc.vector.reciprocal_approx_fast(rc[:], db[:])
                    bcs = small.tile([D, 512], F32, tag="bcs")
                    nc.gpsimd.partition_broadcast(bcs[:], rc[:])
                    nc.vector.tensor_tensor(
                        out=aT_loc[prow : prow + D, acols],
                        in0=bcs[:],
                        in1=psa[0:D, :],
                        op=ALU.mult,
                    )

                # ---- interleaved emission: weave remaining qkv tiles between
                # attention heads so exp (ACT) spreads and PE never starves ----
                qkT_tile(1)
                qkT_tile(5)
                attention_head(0, 0)
                attention_head(0, 1)
                qkT_tile(2)
                qkT_tile(6)
                attention_head(0, 2)
                attention_head(0, 3)
                qkT_tile(3)
                qkT_tile(7)
                attention_head(0, 4)
                attention_head(0, 5)
                v_tile(4)
                v_tile(5)
                attention_head(0, 6)
                attention_head(0, 7)
                gather_half0()
                v_tile(6)
                v_tile(7)
                for h in range(4):
                    attention_head(1, h)
                gather_half1(0)  # ft0/ft1 of qh=1 fly while heads 4-7 compute
                proj_load(0)     # AG#0 result; loads overlap remaining attention
                attention_head(1, 4)
                attention_head(1, 5)
                gather_half1(1)  # ft2 flies while heads 6-7 compute
                proj_load(1, [0, 1, 4, 5])  # prefetch from the early gather
                attention_head(1, 6)
                proj_load(1, [2, 6])        # lands as soon as gather#2 does
                attention_head(1, 7)
                gather_half1(2)

            # ---- c_proj: attention PSUM pools are closed, use a wide pool.
            # Pre-accumulate early-gathered chunks for proj(1); proj(0) tiles
            # keep the PE warm through the final gather's wait; the LATE
            # chunks finish once the last 128 KB gather lands ----
            with tc.tile_pool(name="ps_pj", bufs=6, space="PSUM") as ps_pj:

                def proj_acc(qh2, lt, ps, fcs, first, last):
                    if first:
                        nc.tensor.matmul(
                            ps[:], ones_row[:, 0:P], bp_row[:], start=True, stop=False
                        )
                    for n, fc in enumerate(fcs):
                        nc.tensor.matmul(
                            ps[:],
                            aT_all[
                                :,
                                (qh2 * NKC + fc) * FEAT + lt * P
                                : (qh2 * NKC + fc) * FEAT + (lt + 1) * P,
                            ],
                            wp_sb[:, fc * FEAT : (fc + 1) * FEAT],
                            start=False,
                            stop=(last and n == len(fcs) - 1),
                        )
                    if last:
                        t = 4 * qh2 + lt
                        ot = outp.tile([P, FEAT], F32, tag="ot")
                        if qh2 == 1:
                            nc.vector.tensor_copy(out=ot[:], in_=ps[:])  # ACT-free tail
                            nc.sync.dma_start(out[t * P : (t + 1) * P, :], ot[:])
                        else:
                            nc.scalar.copy(ot[:], ps[:])
                            nc.gpsimd.dma_start(out[t * P : (t + 1) * P, :], ot[:])

                def proj_tile(qh2, lt):
                    ps = ps_pj.tile([P, 512], F32, name="ps_pj0", tag="pj")
                    proj_acc(qh2, lt, ps, list(range(NKC)), True, True)

                EARLY, LATE = [0, 1, 4, 5, 2, 6], [3, 7]
                ps1 = {}
                for lt in range(3):
                    ps1[lt] = ps_pj.tile([P, 512], F32, name=f"ps_p1{lt}", tag="pj")
                    proj_acc(1, lt, ps1[lt], EARLY, True, False)
                for lt in range(4):
                    proj_tile(0, lt)  # fills the final gather's wait
                proj_load(1, [3, 7])
                ps1[3] = ps_pj.tile([P, 512], F32, name="ps_p13", tag="pj")
                proj_acc(1, 3, ps1[3], EARLY, True, False)
                proj_acc(1, 0, ps1[0], LATE, False, True)
                proj_acc(1, 1, ps1[1], LATE, False, True)
                proj_acc(1, 2, ps1[2], LATE, False, True)
                proj_acc(1, 3, ps1[3], LATE, False, True)

    nc.finalize()
    return nc


_NC_CACHE = None
_LAST_IN_MAPS = None


def kernel(x, c_attn_w, c_attn_b, c_proj_w, c_proj_b):
    global _NC_CACHE, _LAST_IN_MAPS
    x = np.asarray(x, dtype=np.float32)
    c_attn_w = np.asarray(c_attn_w, dtype=np.float32)
    c_attn_b = np.asarray(c_attn_b, dtype=np.float32)
    c_proj_w = np.asarray(c_proj_w, dtype=np.float32)
    c_proj_b = np.asarray(c_proj_b, dtype=np.float32)
    B = x.shape[0]
    assert x.shape == (B, S, NX)
    bf16 = ml_dtypes.bfloat16

    xTs = [np.ascontiguousarray(x[b].T).astype(bf16) for b in range(B)]
    in_maps = []
    for c in range(8):
        b, hg = c // 2, c % 2
        cols = slice(hg * FEAT, (hg + 1) * FEAT)
        wq = c_attn_w[:, 0 * NX :][:, cols]
        wk = c_attn_w[:, 1 * NX :][:, cols]
        wv = c_attn_w[:, 2 * NX :][:, cols]
        bq = c_attn_b[0 * NX :][cols]
        bk = c_attn_b[1 * NX :][cols]
        bqk = np.concatenate([bq, bk])                       # [1024]
        in_maps.append(
            {
                "xT": xTs[b],
                "wqkv": np.ascontiguousarray(
                    np.concatenate([wq, wk, c_attn_w[:, 2 * NX :][:, cols]], axis=1)
                ).astype(bf16),
                "bqk_t": np.ascontiguousarray(bqk.reshape(8, P).T),
                "bv_r": np.ascontiguousarray(
                    c_attn_b[2 * NX :][cols].reshape(1, FEAT)
                ).astype(bf16),
                "wproj": np.ascontiguousarray(c_proj_w[:, cols]).astype(bf16),
                "bp_r": np.ascontiguousarray(
                    c_proj_b[cols].reshape(1, FEAT)
                ).astype(bf16),
            }
        )

    _LAST_IN_MAPS = in_maps
    if _NC_CACHE is None:
        _NC_CACHE = build()
    res = run_bass_kernel_spmd(_NC_CACHE, in_maps, core_ids=list(range(8)))
    outf = np.empty((B, S, NX), dtype=np.float32)
    for c in range(8):
        b, hg = c // 2, c % 2
        outf[b, :, hg * FEAT : (hg + 1) * FEAT] = res.results[c]["out"]
    return outf
